# revision 51
# baseline (speedup 1.0000x reference)
"""Trainium2 Bass kernel for a 2-layer GAT (nn_GAT_781684048444).

Strategy (8 NeuronCores, SPMD), v2:
  - Nodes balanced into 80 windows (8 cores x 10 windows x 128 slots) by
    in-degree; edges grouped by dst window, padded to K 128-edge tiles.
  - Stage 0 (replicated): per 128-node block, xh = x @ W1 and the folded
    attention scalars [s_src|s_dst] = x @ Wsd (bf16 in, fp32 acc, one fused
    weight matrix) are packed into a fully initialized 1280B row:
    [msg 512 bf16 | s_src 8 bf | s_dst 8 bf | L2 region 16B | zeros].
    PSUM->SBUF cast copies batched 2/4 blocks and split DVE/Act; the row
    tail past the 32-col scalar block is zero-filled once by a blanket
    gpsimd write; table writes split SP/gpsimd; x, selT loads split.
  - Layer-1 edge phase: fp32 row gathers by src (320 f32) + 256B scalar-
    window gathers by dst; alpha = leakyrelu(s_src + s_dst + ew*c1) with the
    ew x c1 outer product folded on host (bf16, DVE 2x); exp -> bf16;
    msg *= alpha (DVE 2x broadcast multiply, head-minor layout; one thin
    head-slice per window runs on gpsimd to balance engines);
    one-hot selT matmuls accumulate numerator/denominator in
    PSUM; h1 = elu(U/D) in bf16 (min on DVE, exp/relu on Act).
  - Layer-2 projection per window: h1 transposed via DMA-transpose (xbar,
    112ns/chunk on SP/Act), h2aug = h1 @ Waug2 over 4 chunks; compact
    [h2 bf16 x4 | s_src2 | s_dst2] written to table2c (AllGather input) and
    own-node s_dst2 rows into a private tableD; per-window dst gathers
    overlap the remaining layer-1 windows and the collective.
  - Layer 2: 123KB bf16 AllGather ([h2 x4 | s_src2 | pad] per node) ->
    12B/row expansion into the table's L2 region -> 256B gathers; alpha
    batched in 2-window groups pipelined against the gathers; same one-hot
    scatter; batched output write.
"""

import os
import sys

import ml_dtypes
import numpy as np

sys.path.insert(0, "/opt/trn_rl_repo")

from concourse import bacc, bass, mybir, tile  # noqa: E402
from concourse.bass import AP  # noqa: E402
from concourse.bass_utils import run_bass_kernel_spmd  # noqa: E402

N, E = 10000, 160000
IN, HID, OUT, H = 128, 64, 4, 8
C1 = H * HID  # 512 layer-1 out width
NCORES = 8
WIN = 128
NB = 10
NPAD = NB * WIN            # 1280 node slots per core
NWIN = NCORES * NB         # 80 windows
NROW = NCORES * NPAD       # 10240 padded global rows
MAXI = 1024                # dma_gather num_idxs ring limit

ROW = 160                  # table row width in int64 (1280 bytes)
ROWF = ROW * 2             # row width in fp32 units (384)
ROWB = ROW * 4             # row width in bf16 units (768)
# fp32-unit offsets within a row
OFF_SSRC = 256             # s_src 8 f32
OFF_SDST = 264             # s_dst 8 f32
OFF_L2 = 320               # L2 region: [h2 bf16 x4 | s_src2 f32 | s_dst2 f32]
# i64-unit gather offsets
G_SC = 128                 # scalar window (cols 128:160)
G_L2 = 128                 # L2 gather window == scalar window (cols 128:160)

FP = mybir.dt.float32
BF = mybir.dt.bfloat16
I64 = mybir.dt.int64
U64 = mybir.dt.uint64
I16 = mybir.dt.int16

_CACHE = {}

LAST_EXEC_NS = None
LAST_RESULTS = None


def _wrap_idx(vals):
    """int16 gather index layout: idx i -> [i%16, i//16], tiled to 128 partitions."""
    n = vals.shape[0]
    w = np.zeros((16, n // 16), np.int16)
    w[np.arange(n) % 16, np.arange(n) // 16] = vals.astype(np.int16)
    return np.tile(w, (8, 1))


def _build_program(KC, NCH):
    K = KC * NCH
    SL = KC * 128
    POOL_MM = int(os.environ.get("BASS_GAT_POOLMM", "1"))  # msg-mult head-slices on Pool
    XPOSE = os.environ.get("BASS_GAT_XPOSE", "dma")  # dma | pe
    GDT = os.environ.get("BASS_GAT_GDT", "f32")       # i64 | u64 | f32 gather dtype
    GW = 1 if GDT == "f32" else 2                     # i64 units per gather elem
    GTY = {"i64": I64, "u64": U64, "f32": FP}[GDT]

    nc = bacc.Bacc("TRN2", target_bir_lowering=False, debug=False, num_devices=NCORES)

    # ---- DRAM inputs (replicated unless per-core) ----
    xTb_d = nc.dram_tensor("xTb", [IN, NROW], BF, kind="ExternalInput")
    W1b_d = nc.dram_tensor("W1b", [IN, C1 + 32], BF, kind="ExternalInput")
    W2_d = nc.dram_tensor("W2r", [128, 4, 8], BF, kind="ExternalInput")
    ident_d = nc.dram_tensor("ident", [128, 128], BF, kind="ExternalInput")
    # per-core edge data
    srcg_d = nc.dram_tensor("srcg", [128, NB, NCH, KC * 8], I16, kind="ExternalInput")
    dstg_d = nc.dram_tensor("dstg", [128, NB, NCH, KC * 8], I16, kind="ExternalInput")
    dstl_d = nc.dram_tensor("dstl", [128, NB, NCH, KC * 8], I16, kind="ExternalInput")
    ewc8_d = nc.dram_tensor("ewc8", [128, NB, NCH, KC, 8], BF, kind="ExternalInput")
    ew_d = nc.dram_tensor("ew", [128, NB, NCH, KC], FP, kind="ExternalInput")
    selT_d = nc.dram_tensor("selT", [128, NB, NCH, SL], BF, kind="ExternalInput")

    out_d = nc.dram_tensor("out_own", [NPAD, 4], FP, kind="ExternalOutput")

    # ---- internal DRAM ----
    tableM = nc.dram_tensor("tableM", [NROW, ROW], I64)
    tableD = nc.dram_tensor("tableD", [NPAD, 32], I64)       # own L2 dst scalars
    table2c = nc.dram_tensor("table2c", [NPAD, 6], BF)       # AG input
    table2cf = nc.dram_tensor("table2cf", [NROW, 6], BF, addr_space="Shared")

    c2_host = _build_program.c2_host

    with tile.TileContext(nc) as tc:
        with (
            tc.tile_pool(name="const", bufs=1) as constp,
            tc.tile_pool(name="idx", bufs=1) as idxp,
            tc.tile_pool(name="selp", bufs=1) as selp,
            tc.tile_pool(name="gd2", bufs=1) as gd2p,
        ):
            GDT0 = os.environ.get("BASS_GAT_GDT", "f32")
            GW0 = 1 if GDT0 == "f32" else 2
            GTY0 = {"i64": I64, "u64": U64, "f32": FP}[GDT0]
            gdall = gd2p.tile([128, NB, NCH, KC, 32 * 2 // GW0], GTY0)
            gsall = gd2p.tile([128, NB, NCH, KC, 32 * 2 // GW0], GTY0)
            W1b = constp.tile([IN, C1 + 32], BF)
            nc.scalar.dma_start(W1b[:], W1b_d[:])
            W2sb = constp.tile([128, 4, 8], BF)
            nc.scalar.dma_start(W2sb[:], W2_d[:])
            ident = constp.tile([128, 128], BF)
            nc.scalar.dma_start(ident[:], ident_d[:])
            ewc8 = constp.tile([128, NB, NCH, KC, 8], BF)
            nc.sync.dma_start(ewc8[:], ewc8_d[:])
            ewsb = constp.tile([128, NB, NCH, KC], FP)
            nc.sync.dma_start(ewsb[:], ew_d[:])

            srcg = idxp.tile([128, NB, NCH, KC * 8], I16)
            dstg = idxp.tile([128, NB, NCH, KC * 8], I16)
            dstl = idxp.tile([128, NB, NCH, KC * 8], I16)
            selsb = selp.tile([128, NB, NCH, SL], BF)

            # ========== stage 0 (replicated): build tableM rows ==========
            with (
                tc.tile_pool(name="s0", bufs=4) as s0p,
                tc.tile_pool(name="s0x", bufs=1) as s0xp,
                tc.tile_pool(name="s0psA", bufs=3, space="PSUM") as s0psA,
                tc.tile_pool(name="s0psB", bufs=2, space="PSUM") as s0psB,
            ):
                xb = s0xp.tile([IN, NROW], BF, tag="xb")
                for q in range(4):
                    nc.sync.dma_start(xb[:, q * (NROW // 4):(q + 1) * (NROW // 4)],
                                      xTb_d[:, q * (NROW // 4):(q + 1) * (NROW // 4)])
                # edge metadata on gpsimd (Pool is write/gather-bound later)
                nc.gpsimd.dma_start(srcg[:], srcg_d[:])
                nc.gpsimd.dma_start(dstg[:], dstg_d[:])
                nc.gpsimd.dma_start(dstl[:], dstl_d[:])
                nc.scalar.dma_start(selsb[:, 0:NB // 2], selT_d[:, 0:NB // 2])
                nc.sync.dma_start(selsb[:, NB // 2:], selT_d[:, NB // 2:])

                NBLK = NROW // 128  # 80 blocks (padded rows; pad cols are zero)
                ztt = s0xp.tile([128, NBLK, 48], FP, tag="ztt")
                nc.vector.memset(ztt[:], 0.0)
                outT = AP(tableM[:].tensor, 136,
                          [(ROW, 128), (128 * ROW, NBLK), (1, 24)]).bitcast(FP)
                nc.gpsimd.dma_start(outT, ztt[:])
                GRP = 4
                stg = None
                psA2 = psB4 = None
                for b in range(NBLK):
                    j2, j4 = b % 2, b % 4
                    if j2 == 0:
                        psA2 = s0psA.tile([128, 2, C1], FP, tag="psA")
                    if j4 == 0:
                        psB4 = s0psB.tile([128, 4, 32], FP, tag="psB")
                    nc.tensor.matmul(psA2[:, j2, :], xb[:, b * 128:(b + 1) * 128],
                                     W1b[:, 0:C1], start=True, stop=True)
                    nc.tensor.matmul(psB4[:, j4, :], xb[:, b * 128:(b + 1) * 128],
                                     W1b[:, C1:], start=True, stop=True)
                    if b % GRP == 0:
                        stg = s0p.tile([128, GRP, 544], BF, tag="stg")
                    bi = b % GRP
                    # cast copies; GPSIMD cannot read PSUM -> DVE/Act only
                    if j2 == 1:
                        eng = nc.scalar.copy if (b // 2) % 2 == 0 else nc.vector.tensor_copy
                        eng(stg[:, bi - 1:bi + 1, 0:C1], psA2[:])
                    if j4 == 3:
                        eng = nc.vector.tensor_copy if (b // 4) % 2 == 0 else nc.scalar.copy
                        eng(stg[:, :, C1:], psB4[:])
                    if b % GRP == GRP - 1:
                        b0 = b - GRP + 1
                        outM = AP(tableM[:].tensor, b0 * 128 * ROW,
                                  [(ROW, 128), (128 * ROW, GRP), (1, 136)]).bitcast(BF)
                        if (b // GRP) % 2 == 0:
                            nc.sync.dma_start(outM, stg[:])
                        else:
                            nc.gpsimd.dma_start(outM, stg[:])
                ztd = s0p.tile([128, NB, 64], FP, tag="ztd")
                nc.vector.memset(ztd[:], 0.0)
                outD = AP(tableD[:].tensor, 0,
                          [(32, 128), (128 * 32, NB), (1, 32)]).bitcast(FP)
                nc.sync.dma_start(outD, ztd[:])

            # ================= layer 1 edge phase + L2 projection =================
            with (
                tc.tile_pool(name="g1", bufs=4) as g1p,
                tc.tile_pool(name="gd1", bufs=4) as gd1p,
                tc.tile_pool(name="al1", bufs=4) as al1p,
                tc.tile_pool(name="wend", bufs=2) as wendp,
                tc.tile_pool(name="l2h", bufs=2) as l2hp,
                tc.tile_pool(name="ps1", bufs=3, space="PSUM") as ps1p,
                tc.tile_pool(name="psd", bufs=2, space="PSUM") as psdp,
                tc.tile_pool(name="l2ps", bufs=(2 if XPOSE == "dma" else 1), space="PSUM") as l2ps,
            ):
                st2c = wendp.tile([128, NB, 6], BF, tag="st2c")
                std = wendp.tile([128, NB, 2], FP, tag="std")
                nc.vector.memset(st2c[:], 0.0)
                nc.vector.memset(std[:], 0.0)
                for w in range(NB):
                    psU = ps1p.tile([128, C1], FP)
                    psD = psdp.tile([128, 8], FP)
                    for ch in range(NCH):
                        g = g1p.tile([128, KC, 160 * 2 // GW], GTY)
                        tmv = tableM[:].bitcast(GTY) if GDT != "i64" else tableM[:]
                        nc.gpsimd.dma_gather(
                            g[:], tmv[:, 0:160 * 2 // GW], srcg[:, w, ch, :],
                            SL, SL, 160 * 2 // GW, elem_step=ROW * 2 // GW,
                        )
                        gd = gd1p.tile([128, KC, 32 * 2 // GW], GTY)
                        nc.gpsimd.dma_gather(
                            gd[:], tmv[:, G_SC * 2 // GW:(G_SC + 32) * 2 // GW],
                            dstg[:, w, ch, :],
                            SL, SL, 32 * 2 // GW, elem_step=ROW * 2 // GW,
                        )
                        gb0 = g[:].bitcast(BF)     # [128, KC, 640]
                        gdb = gd[:].bitcast(BF)    # [128, KC, 128]
                        # alpha = s_src[src] + s_dst[dst] + ew*c1 (bf16, 2x)
                        a = al1p.tile([128, KC, 8], BF)
                        nc.vector.tensor_tensor(
                            out=a[:], in0=gb0[:, :, 512:520], in1=gdb[:, :, 8:16],
                            op=mybir.AluOpType.add,
                        )
                        nc.vector.tensor_tensor(
                            out=a[:], in0=a[:], in1=ewc8[:, w, ch, :, :],
                            op=mybir.AluOpType.add,
                        )
                        nc.vector.scalar_tensor_tensor(
                            out=a[:], in0=a[:], scalar=0.2, in1=a[:],
                            op0=mybir.AluOpType.mult, op1=mybir.AluOpType.max)
                        ahb = al1p.tile([128, KC, 1, 8], BF)
                        nc.scalar.activation(ahb[:, :, 0, :], a[:],
                                             mybir.ActivationFunctionType.Exp)
                        # msg *= alpha-hat (head-minor layout -> stride-1 2x mode)
                        gb = g[:].bitcast(BF)      # [128, KC, 640]
                        msg4 = gb[:, :, 0:512].rearrange("p t (c h) -> p t c h", h=8)
                        ah4 = ahb[:].to_broadcast([128, KC, 64, 8])
                        if POOL_MM > 0 and ch == 0:
                            # thin head-slice on gpsimd, first chunk per window
                            hs = POOL_MM if POOL_MM < 8 else 1
                            nc.gpsimd.tensor_tensor(
                                out=msg4[:, :, :, 0:hs], in0=msg4[:, :, :, 0:hs],
                                in1=ahb[:, :, :, 0:hs].to_broadcast([128, KC, 64, hs]),
                                op=mybir.AluOpType.mult)
                            nc.vector.tensor_tensor(
                                out=msg4[:, :, :, hs:8], in0=msg4[:, :, :, hs:8],
                                in1=ahb[:, :, :, hs:8].to_broadcast([128, KC, 64, 8 - hs]),
                                op=mybir.AluOpType.mult)
                        else:
                            nc.vector.tensor_tensor(out=msg4, in0=msg4, in1=ah4,
                                                    op=mybir.AluOpType.mult)
                        for t in range(KC):
                            ti = ch * KC + t
                            st = ti == 0
                            sp = ti == K - 1
                            sel = selsb[:, w, ch, t * 128:(t + 1) * 128]
                            nc.tensor.matmul(psU[:], sel, gb[:, t, 0:512],
                                             start=st, stop=sp)
                            nc.tensor.matmul(psD[:], sel, ahb[:, t, 0, :],
                                             start=st, stop=sp)
                    # ---- window finalize: h1 = elu(U/D) (b1 == 0) ----
                    dpe = wendp.tile([128, 8], FP, tag="dpe")
                    nc.vector.tensor_scalar_add(dpe[:], psD[:], 1e-16)
                    dr = wendp.tile([128, 1, 8], FP, tag="dr")
                    nc.vector.reciprocal(dr[:, 0, :], dpe[:])
                    h1 = wendp.tile([128, C1], BF, tag="h1")
                    h13 = h1[:].rearrange("p (c h) -> p c h", h=8)
                    psU3 = psU[:].rearrange("p (c h) -> p c h", h=8)
                    nc.vector.tensor_tensor(out=h13, in0=psU3,
                                            in1=dr[:].to_broadcast([128, 64, 8]),
                                            op=mybir.AluOpType.mult)
                    tmin = wendp.tile([128, C1], BF, tag="tmin")
                    nc.vector.tensor_scalar_min(tmin[:], h1[:], 0.0)
                    nc.scalar.activation(tmin[:], tmin[:],
                                         mybir.ActivationFunctionType.Exp)
                    nc.scalar.activation(h1[:], h1[:],
                                         mybir.ActivationFunctionType.Relu)
                    nc.vector.scalar_tensor_tensor(
                        out=h1[:], in0=h1[:], scalar=-1.0, in1=tmin[:],
                        op0=mybir.AluOpType.add, op1=mybir.AluOpType.add,
                    )
                    # ---- layer-2 projection: h2aug = h1 @ Waug2 ----
                    h1T = l2hp.tile([128, 4, 128], BF, tag="h1T")
                    if XPOSE == "dma":
                        for kc in range(4):
                            (nc.sync if kc % 2 == 0 else nc.scalar).dma_start_transpose(
                                h1T[:, kc, :], h1[:, kc * 128:(kc + 1) * 128])
                    else:
                        for kc in range(4):
                            pst = l2ps.tile([128, 128], BF, tag="pst")
                            nc.tensor.transpose(pst[:], h1[:, kc * 128:(kc + 1) * 128],
                                                ident[:])
                            (nc.vector.tensor_copy if kc % 2 else nc.scalar.copy)(
                                h1T[:, kc, :], pst[:])
                    ps2 = l2ps.tile([128, 8], FP)
                    for kc in range(4):
                        nc.tensor.matmul(ps2[:], h1T[:, kc, :], W2sb[:, kc, :],
                                         start=(kc == 0), stop=(kc == 3))
                    # compact rows: [h2 bf16 x4 | s_src2 f32]; dst scalar separate
                    nc.vector.tensor_copy(st2c[:, w, 0:4], ps2[:, 0:4])
                    nc.vector.tensor_copy(st2c[:, w, 4:5], ps2[:, 4:5])
                    nc.vector.tensor_copy(std[:, w, 0:1], ps2[:, 5:6])
                    # own dst scalars for this window -> tableD, then dst gathers
                    nc.sync.dma_start(
                        AP(tableD[:].tensor, w * 128 * 32,
                           [(32, 128), (1, 1)]).bitcast(FP),
                        std[:, w, :])
                    for ch in range(NCH):
                        tdv = tableD[:].bitcast(GTY) if GDT != "i64" else tableD[:]
                        nc.gpsimd.dma_gather(
                            gdall[:, w, ch], tdv, dstl[:, w, ch, :], SL, SL,
                            32 * 2 // GW,
                        )
                # AG input, then AllGather
                nc.sync.dma_start(
                    AP(table2c[:].tensor, 0, [(6, 128), (128 * 6, NB), (1, 6)]),
                    st2c[:])
                cceng = {"pool": nc.gpsimd, "pe": nc.tensor, "act": nc.scalar,
                         "dve": nc.vector}[os.environ.get("BASS_GAT_CCENG", "pool")]
                bass.BassGpSimd.collective_compute(
                    cceng,
                    "AllGather",
                    mybir.AluOpType.bypass,
                    replica_groups=[list(range(NCORES))],
                    ins=[table2c[:]],
                    outs=[table2cf[:]],
                )
                # expand AG rows into the tableM L2 region (16B per row)
                outX = tableM[:].bitcast(BF)[:, 528:534]
                nc.sync.dma_start(outX, table2cf[:])

            # ================= layer 2 edge phase =================
            with (
                tc.tile_pool(name="al2", bufs=2) as al2p,
                tc.tile_pool(name="wend2", bufs=2) as wend2p,
                tc.tile_pool(name="ps2p", bufs=2, space="PSUM") as ps2pp,
            ):
                tmv2 = tableM[:].bitcast(GTY) if GDT != "i64" else tableM[:]
                gsf = gsall[:].bitcast(FP)   # [128, NB, NCH, KC, 64]
                gdf = gdall[:].bitcast(FP)
                gsb = gsall[:].bitcast(BF)   # [128, NB, NCH, KC, 128]
                HN = NB // 5
                for h in range(5):
                    ws = slice(h * HN, (h + 1) * HN)
                    for w in range(h * HN, (h + 1) * HN):
                        for ch in range(NCH):
                            nc.gpsimd.dma_gather(
                                gsall[:, w, ch], tmv2[:, G_L2 * 2 // GW:(G_L2 + 32) * 2 // GW],
                                srcg[:, w, ch, :], SL, SL, 32 * 2 // GW,
                                elem_step=ROW * 2 // GW,
                            )
                    # half-batched alpha2 = s_src2[src] + s_dst2[dst] + ew*c2
                    a2 = al2p.tile([128, HN, NCH, KC, 1], FP, tag="a2")
                    nc.vector.tensor_tensor(out=a2[:], in0=gsb[:, ws, :, :, 20:21],
                                            in1=gdf[:, ws, :, :, 0:1],
                                            op=mybir.AluOpType.add)
                    ew_b = ewsb[:, ws].rearrange("p w c (t o) -> p w c t o", o=1)
                    nc.vector.scalar_tensor_tensor(
                        out=a2[:], in0=ew_b, scalar=float(c2_host), in1=a2[:],
                        op0=mybir.AluOpType.mult, op1=mybir.AluOpType.add,
                    )
                    nc.vector.scalar_tensor_tensor(
                        out=a2[:], in0=a2[:], scalar=0.2, in1=a2[:],
                        op0=mybir.AluOpType.mult, op1=mybir.AluOpType.max)
                    nc.scalar.activation(gsb[:, ws, :, :, 20:21], a2[:],
                                         mybir.ActivationFunctionType.Exp)
                    ah = gsb[:, ws, :, :, 20:21].to_broadcast([128, HN, NCH, KC, 4])
                    nc.vector.tensor_tensor(out=gsb[:, ws, :, :, 16:20],
                                            in0=gsb[:, ws, :, :, 16:20],
                                            in1=ah, op=mybir.AluOpType.mult)
                oball = wend2p.tile([128, NB, 4], FP, tag="oball")
                for w in range(NB):
                    psO = ps2pp.tile([128, 8], FP)
                    for ch in range(NCH):
                        for t in range(KC):
                            ti = ch * KC + t
                            sel = selsb[:, w, ch, t * 128:(t + 1) * 128]
                            nc.tensor.matmul(psO[:, 0:5], sel, gsb[:, w, ch, t, 16:21],
                                             start=(ti == 0), stop=(ti == K - 1))
                    dpe = wend2p.tile([128, 1], FP, tag="dpe2")
                    nc.vector.tensor_scalar_add(dpe[:], psO[:, 4:5], 1e-16)
                    dr = wend2p.tile([128, 1], FP, tag="dr2")
                    nc.vector.reciprocal(dr[:], dpe[:])
                    nc.vector.tensor_tensor(out=oball[:, w, :], in0=psO[:, 0:4],
                                            in1=dr[:].to_broadcast([128, 4]),
                                            op=mybir.AluOpType.mult)
                nc.sync.dma_start(
                    AP(out_d[:].tensor, 0, [(4, 128), (128 * 4, NB), (1, 4)]),
                    oball[:])

    nc.compile()
    return nc


def _balance_windows(dst):
    """Greedy in-degree balancing of nodes into NWIN windows of WIN slots."""
    import heapq

    indeg = np.bincount(dst, minlength=N)
    order = np.argsort(-indeg, kind="stable")
    heap = [(0, w) for w in range(NWIN)]
    heapq.heapify(heap)
    fill = np.zeros(NWIN, np.int64)
    node_win = np.zeros(N, np.int64)
    node_slot = np.zeros(N, np.int64)
    for n in order:
        cnt, w = heapq.heappop(heap)
        node_win[n] = w
        node_slot[n] = fill[w]
        fill[w] += 1
        if fill[w] < WIN:
            heapq.heappush(heap, (cnt + int(indeg[n]), w))
    return node_win, node_slot


def _prepare(x, edge_index, edge_weight, W1, att_src1, att_dst1, att_edge1, We1, b1,
             W2, att_src2, att_dst2, att_edge2, We2, b2):
    x = np.asarray(x, np.float32)
    ei = np.asarray(edge_index)
    ew = np.asarray(edge_weight, np.float32)
    W1 = np.asarray(W1, np.float32)
    att_src1 = np.asarray(att_src1, np.float32)
    att_dst1 = np.asarray(att_dst1, np.float32)
    att_edge1 = np.asarray(att_edge1, np.float32)
    We1 = np.asarray(We1, np.float32)
    b1 = np.asarray(b1, np.float32)
    W2 = np.asarray(W2, np.float32)
    att_src2 = np.asarray(att_src2, np.float32)
    att_dst2 = np.asarray(att_dst2, np.float32)
    att_edge2 = np.asarray(att_edge2, np.float32)
    We2 = np.asarray(We2, np.float32)
    b2 = np.asarray(b2, np.float32)
    assert not np.any(b1) and not np.any(b2), "nonzero biases unsupported"

    # ---------- weight folding (host, weights only) ----------
    W1r = W1.reshape(IN, H, HID)
    Wsrc = np.einsum("khc,hc->kh", W1r, att_src1)
    Wdst = np.einsum("khc,hc->kh", W1r, att_dst1)
    Wsd = np.concatenate(
        [Wsrc, Wdst, np.zeros((IN, 32 - 2 * H), np.float32)], axis=1)
    c1 = (We1.reshape(H, HID) * att_edge1).sum(1).astype(np.float32)  # [H]

    # head-minor column order: new col c*8+h = old h*64+c
    cols = np.tile(np.arange(H), HID) * HID + np.repeat(np.arange(HID), H)
    W1p = np.ascontiguousarray(W1[:, cols])
    W2p = W2[cols, :]

    Waug2 = np.zeros((C1, 8), np.float32)
    Waug2[:, 0:4] = W2p
    Waug2[:, 4] = W2p @ att_src2[0]
    Waug2[:, 5] = W2p @ att_dst2[0]
    W2resh = np.ascontiguousarray(Waug2.reshape(4, 128, 8).transpose(1, 0, 2))
    c2 = float((We2[0] * att_edge2[0]).sum())
    _build_program.c2_host = c2

    # ---------- node/edge partitioning ----------
    src = np.asarray(ei[0], np.int64)
    dst = np.asarray(ei[1], np.int64)

    node_win, node_slot = _balance_windows(dst)
    node_core = node_win // NB
    node_w = node_win % NB
    node_local = node_w * WIN + node_slot
    node_gpad = node_core * NPAD + node_local

    # x columns permuted into padded-global order (pad cols zero)
    xTp = np.zeros((IN, NROW), np.float32)
    xTp[:, node_gpad] = x.T

    ekey = node_win[dst]
    order = np.argsort(ekey, kind="stable")
    s_s, d_s, w_s = src[order], dst[order], ew[order]
    core_of = node_core[d_s]
    win_of = node_w[d_s]
    loc_of = node_slot[d_s]

    cnt = np.bincount(node_win[d_s], minlength=NWIN)
    K = int(np.ceil(cnt.max() / 128.0))
    NCH = 2
    while ((K + NCH - 1) // NCH) * 128 > MAXI:
        NCH += 1
    KC = (K + NCH - 1) // NCH
    K = KC * NCH
    SL = KC * 128
    SW = K * 128

    in_maps = []
    base_rep = {
        "xTb": xTp.astype(ml_dtypes.bfloat16),
        "W1b": np.concatenate([W1p, Wsd], axis=1).astype(ml_dtypes.bfloat16),
        "W2r": W2resh.astype(ml_dtypes.bfloat16),
        "ident": np.eye(128, dtype=np.float32).astype(ml_dtypes.bfloat16),
    }

    for c in range(NCORES):
        m = dict(base_rep)
        srcg = np.zeros((NB, NCH, 128, KC * 8), np.int16)
        dstg = np.zeros((NB, NCH, 128, KC * 8), np.int16)
        dstl = np.zeros((NB, NCH, 128, KC * 8), np.int16)
        ews = np.zeros((NB, NCH, KC, 128), np.float32)
        ewc8 = np.zeros((NB, NCH, KC, 128, 8), np.float32)
        selT = np.zeros((NB, NCH, 128, SL), np.float32)
        sel_c = core_of == c
        for w in range(NB):
            es = np.nonzero(sel_c & (win_of == w))[0]
            ns = len(es)
            ssrc = np.zeros(SW, np.int64)
            sdst = np.zeros(SW, np.int64)
            sew = np.zeros(SW, np.float32)
            sloc = np.full(SW, -1, np.int64)
            ssrc[:ns] = node_gpad[s_s[es]]
            sdst[:ns] = node_gpad[d_s[es]]
            sew[:ns] = w_s[es]
            sloc[:ns] = loc_of[es]
            for ch in range(NCH):
                sl = slice(ch * SL, (ch + 1) * SL)
                srcg[w, ch] = _wrap_idx(ssrc[sl])
                dstg[w, ch] = _wrap_idx(sdst[sl])
                dstl[w, ch] = _wrap_idx(np.where(sloc[sl] >= 0, sdst[sl] - c * NPAD, 0))
                ews[w, ch] = sew[sl].reshape(KC, 128)
                ewc8[w, ch] = sew[sl].reshape(KC, 128)[:, :, None] * c1[None, None, :]
                lc = sloc[sl]
                valid = np.nonzero(lc >= 0)[0]
                tt, pp = valid // 128, valid % 128
                selT[w, ch, pp, tt * 128 + lc[valid]] = 1.0
        m["srcg"] = np.ascontiguousarray(srcg.transpose(2, 0, 1, 3))
        m["dstg"] = np.ascontiguousarray(dstg.transpose(2, 0, 1, 3))
        m["dstl"] = np.ascontiguousarray(dstl.transpose(2, 0, 1, 3))
        m["ew"] = np.ascontiguousarray(ews.transpose(3, 0, 1, 2))
        m["ewc8"] = np.ascontiguousarray(
            ewc8.transpose(3, 0, 1, 2, 4)).astype(ml_dtypes.bfloat16)
        m["selT"] = np.ascontiguousarray(
            selT.transpose(2, 0, 1, 3)).astype(ml_dtypes.bfloat16)
        in_maps.append(m)

    meta = (node_core, node_local)
    return in_maps, KC, NCH, c2, meta


def kernel(**inputs):
    global LAST_EXEC_NS, LAST_RESULTS
    in_maps, KC, NCH, c2, meta = _prepare(**inputs)
    key = (KC, NCH, c2)
    if key not in _CACHE:
        _CACHE[key] = _build_program(KC, NCH)
    nc = _CACHE[key]

    trace = os.environ.get("BASS_GAT_TRACE", "0") == "1"
    res = run_bass_kernel_spmd(nc, in_maps, list(range(NCORES)), trace=trace)
    LAST_EXEC_NS = res.exec_time_ns
    LAST_RESULTS = res
    node_core, node_local = meta
    per_core = [res.results[c]["out_own"] for c in range(NCORES)]
    out = np.empty((N, 4), np.float32)
    for c in range(NCORES):
        mask = node_core == c
        out[mask] = per_core[c][node_local[mask]]
    return out
